# revision 7
# baseline (speedup 1.0000x reference)
"""BiDAF attention-flow kernel for Trainium2 (Bass/Tile), SPMD over 8 cores.

Math (per batch element b, one NeuronCore each):
    cq[c,j] = sum_h e2[c,h] * wcq[h] * e1[j,h]
    s[c,j]  = sc[c] + sq[j] + cq[c,j]            (+ scalar biases, which
                                                  cancel in both softmaxes)
    a       = softmax_j(s)
    c2q     = a @ e1                              (B,C,H)
    b_att   = softmax_c(max_j s)
    q2c     = b_att @ e2                          (H,)
    out     = [e2, c2q, e2*c2q, e2*q2c] @ w_red.T + b_red

Device layout: everything lives transposed, [h on partitions, c free]:
    sT[j,c] (PSUM) -> P_T = exp(sT + sq[j])      (unnormalized; row max not
                                                  subtracted - fp32 range is
                                                  plenty for |s| <= ~12)
    L[c] = sum_j P_T  via ones-matmul            a = P_T / L
    c2qT[h,c] = e1.T @ P_T, scaled by 1/L at PSUM eviction
    max_j s   = partition_all_reduce-max of max-over-jt-tiles of P_T
                (exp is monotone), so E = M*exp(sc) with no transposes, and
    q2c = (sum_c E[c]*e2T[:,c]) / sum_c E[c]     accumulated unnormalized
                                                  while the PE works
    q2c folded into the last 6 k-tiles of w_red: wq4T[h,:] = q2c[h]*wrT[18+ht]

Phases (each fully unrolled; Tile pipelines across them):
    A: cq matmuls (jt-outer, ht-mid, ch-inner for stationary-weight reuse)
       + exp + running max + L ones-matmuls + sc rows
    B: c2q matmuls (ht-outer, jt-mid, ch-inner, stationary reuse),
       1/L eviction scaling; partition_all_reduce + E row in parallel
    C: reduction-layer pass 1 (k-tiles 0..17) + interleaved q2c mul-reduces
    D: pass 2 (k-tiles 18..23 + bias), add, store

Host does sharding/layout only: batch split, transposes, bf16 casts.
"""

import numpy as np
import ml_dtypes

B, Q, C, H, OUT = 8, 512, 2048, 768, 300
HT, JT, CT = H // 128, Q // 128, C // 128  # 6, 4, 16
NCH, CHW = 4, 512  # c chunks
CPT = 4  # c-tiles per chunk

bf16 = ml_dtypes.bfloat16

_CACHE = {}


def _build_bass():
    import concourse.tile as tile
    from concourse import mybir, bass_isa, library_config, bacc

    f32 = mybir.dt.float32
    b16 = mybir.dt.bfloat16
    AF = mybir.ActivationFunctionType
    OP = mybir.AluOpType

    f8 = mybir.dt.float8e4
    DR = mybir.MatmulPerfMode.DoubleRow

    nc = bacc.Bacc("TRN2", target_bir_lowering=False, debug=False)

    e1_d = nc.dram_tensor("e1", [Q, H], b16, kind="ExternalInput").ap()
    e1t_d = nc.dram_tensor("e1t", [H, Q], b16, kind="ExternalInput").ap()
    e2t_d = nc.dram_tensor("e2t", [H, C], b16, kind="ExternalInput").ap()
    e1wh_d = nc.dram_tensor("e1wh", [H, Q], f8, kind="ExternalInput").ap()
    e1wl_d = nc.dram_tensor("e1wl", [H, Q], f8, kind="ExternalInput").ap()
    e2th_d = nc.dram_tensor("e2th", [H, C], f8, kind="ExternalInput").ap()
    e2tl_d = nc.dram_tensor("e2tl", [H, C], f8, kind="ExternalInput").ap()
    wrt_d = nc.dram_tensor("wrt", [4 * H, OUT], b16, kind="ExternalInput").ap()
    wpk_d = nc.dram_tensor("wpk", [128, 3 * HT], f32, kind="ExternalInput").ap()
    bred_d = nc.dram_tensor("bred", [1, OUT], b16, kind="ExternalInput").ap()
    out_d = nc.dram_tensor("out", [C, OUT], f32, kind="ExternalOutput").ap()

    with tile.TileContext(nc) as tc:
        with (
            tc.tile_pool(name="singles", bufs=1) as singles,
            tc.tile_pool(name="m3", bufs=8) as m3p,
            tc.tile_pool(name="odma", bufs=4) as odp,
            tc.tile_pool(name="ps_mm", bufs=6, space="PSUM") as ps_mm,
            tc.tile_pool(name="ps_out", bufs=2, space="PSUM") as ps_out,
        ):
            # gpsimd: need the 'attn' ucode library for partition_all_reduce
            nc.gpsimd.load_library(library_config.attn)

            # ---- persistent SBUF tensors -------------------------------
            e1_sb = singles.tile([128, JT, H], b16)      # emb1, j on parts
            e1t_sb = singles.tile([128, HT, Q], b16)     # emb1.T, h on parts
            e1wh_sb = singles.tile([128, HT, Q], f8)     # fp8 hi of 16*wcq*e1T
            e1wl_sb = singles.tile([128, HT, Q], f8)     # fp8 lo residual
            e2th_sb = singles.tile([128, HT, C], f8)     # fp8 hi of e2T
            e2tl_sb = singles.tile([128, HT, C], f8)     # fp8 lo residual
            e2t_sb = singles.tile([128, HT, C], b16)     # emb2.T, h on parts
            wrt_sb = singles.tile([128, 24, OUT], b16)   # w_red.T, f on parts
            wq4_sb = singles.tile([128, HT, OUT], b16)   # q2c-folded wrT tail
            wsum_sb = singles.tile([128, HT, OUT], b16)  # wrT[0:6] + wq4T
            wpk_sb = singles.tile([128, 3 * HT], f32)
            wq_sb = singles.tile([128, HT], b16)
            bred_sb = singles.tile([1, OUT], b16)
            ones_mat = singles.tile([128, 128], b16)
            ones_row_b = singles.tile([1, 128], b16)
            ones_row_f = singles.tile([1, 128], f32)
            sq_sb = singles.tile([128, JT], f32)         # sq as columns
            escb_sb = singles.tile([128, C], b16)        # exp(sc) bcast
            wc_mat = singles.tile([128, HT, 128], b16)   # wc[h] rank-1 bcast
            pt_sb = singles.tile([128, JT, NCH, CHW], b16)  # P_T = exp(sT+sq)
            c2q_sb = singles.tile([128, HT, C], b16)     # c2qT (normalized)
            macc = singles.tile([128, C], b16)           # col-max of P_T
            mall = singles.tile([128, C], b16)           # after all-reduce
            ebc_sb = singles.tile([128, C], b16)         # E bcast over parts
            s_parts = singles.tile([1, NCH], f32)
            s_sum = singles.tile([1, 1], f32)
            rs_sum = singles.tile([1, 1], f32)
            rs_col = singles.tile([128, 1], f32)
            bcr_sb = singles.tile([128, C], f32)         # 1/L bcast over parts
            u_sb = singles.tile([128, HT, NCH], f32)     # unnormalized q2c
            q2c_sb = singles.tile([128, HT], f32)
            out_sb = singles.tile([128, CT, OUT], f32)   # pass-1 partials

            # ---- loads (ordered for earliest PE start) -----------------
            nc.sync.dma_start(
                out=e1wh_sb, in_=e1wh_d.rearrange("(t p) j -> p t j", p=128)
            )
            nc.sync.dma_start(
                out=e1wl_sb, in_=e1wl_d.rearrange("(t p) j -> p t j", p=128)
            )
            e2th_r = e2th_d.rearrange("(t p) c -> p t c", p=128)
            e2tl_r = e2tl_d.rearrange("(t p) c -> p t c", p=128)
            for hf in range(2):
                fsl = slice(hf * 2 * CHW, (hf + 1) * 2 * CHW)
                for ht in range(HT):
                    nc.sync.dma_start(out=e2th_sb[:, ht, fsl], in_=e2th_r[:, ht, fsl])
                for ht in range(HT):
                    nc.sync.dma_start(out=e2tl_sb[:, ht, fsl], in_=e2tl_r[:, ht, fsl])
            nc.sync.dma_start(
                out=e1t_sb, in_=e1t_d.rearrange("(t p) j -> p t j", p=128)
            )
            nc.sync.dma_start(out=wpk_sb, in_=wpk_d)
            nc.vector.memset(ones_mat, 1.0)
            nc.vector.memset(ones_row_b, 1.0)
            nc.vector.memset(ones_row_f, 1.0)
            nc.vector.memset(macc, 0.0)
            nc.vector.tensor_copy(wq_sb, wpk_sb[:, 2 * HT : 3 * HT])
            for ht in range(HT):
                nc.vector.tensor_scalar_mul(
                    wc_mat[:, ht, :], ones_mat,
                    wpk_sb[:, HT + ht : HT + ht + 1],
                )
            # HAM warm-up: keep the PE busy while inputs stream in, so the
            # clock gate is at 8/8 when the real matmuls start
            wps = ps_mm.tile([128, CHW], f32, tag="mm", name="warm")
            for _ in range(70):
                nc.tensor.matmul(wps[:, 0:128], ones_mat, ones_mat,
                                 start=True, stop=True)
            nc.vector.tensor_copy(rs_col, wps[:, 0:1])
            e2t_r = e2t_d.rearrange("(t p) c -> p t c", p=128)
            for hf in range(2):
                fsl = slice(hf * 2 * CHW, (hf + 1) * 2 * CHW)
                for ht in range(HT):
                    nc.sync.dma_start(
                        out=e2t_sb[:, ht, fsl], in_=e2t_r[:, ht, fsl]
                    )
            nc.sync.dma_start(
                out=e1_sb, in_=e1_d.rearrange("(t p) h -> p t h", p=128)
            )
            nc.sync.dma_start(
                out=wrt_sb, in_=wrt_d.rearrange("(t p) o -> p t o", p=128)
            )
            nc.sync.dma_start(out=bred_sb, in_=bred_d)

            # ---- sq columns (tiny, feeds exp bias) ---------------------
            for jt in range(JT):
                ps = ps_mm.tile([128, CHW], f32, tag="mm")
                for ht in range(HT):
                    nc.tensor.matmul(
                        ps[:, 0:1],
                        e1t_sb[:, ht, jt * 128 : (jt + 1) * 128],
                        wq_sb[:, ht : ht + 1],
                        start=(ht == 0),
                        stop=(ht == HT - 1),
                    )
                nc.vector.tensor_copy(sq_sb[:, jt : jt + 1], ps[:, 0:1])

            # ---- phase A: sT matmuls (fp8 hi/lo DoubleRow), exp, max ---
            # jt outer / ht-pair mid / hi-lo combo / ch inner: stationary
            # tile reused across the 4 chunks. PSUM gets
            # 16*cq = sum of hh + hl + lh contributions (lo*lo dropped).
            A_COMBOS = [(e1wh_sb, e2th_sb), (e1wh_sb, e2tl_sb), (e1wl_sb, e2th_sb)]
            for jt in range(JT):
                sps = [
                    ps_mm.tile([128, CHW], f32, tag="mm", name=f"sps{jt}_{i}")
                    for i in range(NCH)
                ]
                jsl = slice(jt * 128, (jt + 1) * 128)
                for htp in range(HT // 2):
                    hsl2 = slice(2 * htp, 2 * htp + 2)
                    for ci, (st, mv) in enumerate(A_COMBOS):
                        for ch in range(NCH):
                            nc.tensor.matmul(
                                sps[ch],
                                st[:, hsl2, jsl],
                                mv[:, hsl2, ch * CHW : (ch + 1) * CHW],
                                start=(htp == 0 and ci == 0),
                                stop=(htp == HT // 2 - 1 and ci == 2),
                                perf_mode=DR,
                            )
                for ch in range(NCH):
                    csl = slice(ch * CHW, (ch + 1) * CHW)
                    nc.scalar.activation(
                        out=pt_sb[:, jt, ch, :], in_=sps[ch], func=AF.Exp,
                        bias=sq_sb[:, jt : jt + 1], scale=1.0 / 16.0,
                    )
                    nc.vector.tensor_max(
                        macc[:, csl], macc[:, csl], pt_sb[:, jt, ch, :]
                    )

            # ---- 1/L, already broadcast across partitions --------------
            # ones-matrix stationary: out[m,c] = sum_j P_T[j,c] for every m,
            # i.e. L[c] replicated on all 128 partitions, in one matmul per
            # (jt, chunk); then a fast reciprocal straight into bcr.
            for ch in range(NCH):
                csl = slice(ch * CHW, (ch + 1) * CHW)
                lps = ps_mm.tile([128, CHW], f32, tag="mm", name=f"lps{ch}")
                for jt in range(JT):
                    nc.tensor.matmul(
                        lps, ones_mat, pt_sb[:, jt, ch, :],
                        start=(jt == 0), stop=(jt == JT - 1),
                    )
                nc.vector.reciprocal_approx_fast(out=bcr_sb[:, csl], in_=lps)

            # ---- exp(sc), broadcast across partitions (rank-1 weights) -
            for ch in range(NCH):
                csl = slice(ch * CHW, (ch + 1) * CHW)
                ps = ps_mm.tile([128, CHW], f32, tag="mm")
                for ht in range(HT):
                    nc.tensor.matmul(
                        ps,
                        wc_mat[:, ht, :],
                        e2t_sb[:, ht, csl],
                        start=(ht == 0),
                        stop=(ht == HT - 1),
                    )
                nc.scalar.activation(
                    out=escb_sb[:, csl], in_=ps, func=AF.Exp,
                    bias=0.0, scale=1.0,
                )

            # ---- phase B: c2qT matmuls with 1/L eviction scaling -------
            # ht outer / jt mid / ch inner: e1 stationary tile reused
            # across the 4 chunks (one LDWEIGHTS per (ht, jt)).
            for ht in range(HT):
                cps = [
                    ps_mm.tile([128, CHW], f32, tag="mm", name=f"cps{ht}_{i}")
                    for i in range(NCH)
                ]
                for jt in range(JT):
                    for ch in range(NCH):
                        nc.tensor.matmul(
                            cps[ch],
                            e1_sb[:, jt, ht * 128 : (ht + 1) * 128],
                            pt_sb[:, jt, ch, :],
                            start=(jt == 0),
                            stop=(jt == JT - 1),
                        )
                for ch in range(NCH):
                    csl = slice(ch * CHW, (ch + 1) * CHW)
                    nc.vector.tensor_mul(
                        c2q_sb[:, ht, csl], cps[ch], bcr_sb[:, csl]
                    )

            # ---- b_att numerator: all-reduce max, E row, E bcast -------
            nc.gpsimd.partition_all_reduce(
                mall, macc, channels=128, reduce_op=bass_isa.ReduceOp.max
            )
            for ch in range(NCH):
                csl = slice(ch * CHW, (ch + 1) * CHW)
                nc.vector.tensor_mul(
                    ebc_sb[:, csl], mall[:, csl], escb_sb[:, csl]
                )
                nc.vector.reduce_sum(
                    out=s_parts[:, ch : ch + 1], in_=ebc_sb[0:1, csl],
                    axis=mybir.AxisListType.X,
                )

            # ---- phase C/D: reduction pass 1, q2c work, pass 2 ---------
            # pass-2 for chunk ch-1 is emitted after pass-1 of chunk ch so
            # the PE never waits on wq4 (ready while chunk 0/1 pass-1 runs),
            # and output DMAs spread across the tail.
            def pass2(ch):
                for lc in range(CPT):
                    ct = ch * CPT + lc
                    tsl = slice(ct * 128, (ct + 1) * 128)
                    obs = ps_out.tile([128, OUT], f32, tag="out", name=f"obs{ct}")
                    for ht in range(HT):
                        nc.tensor.matmul(
                            obs, e2t_sb[:, ht, tsl], wq4_sb[:, ht, :],
                            start=(ht == 0), stop=False,
                        )
                    nc.tensor.matmul(
                        obs, ones_row_b, bred_sb, start=False, stop=True,
                    )
                    od = odp.tile([128, OUT], f32, tag="od", name=f"od{ct}")
                    nc.vector.tensor_add(od, obs, out_sb[:, ct, :])
                    nc.sync.dma_start(out=out_d[tsl, :], in_=od)

            def pass1(ch):
                csl = slice(ch * CHW, (ch + 1) * CHW)
                m3s = []
                for ht in range(HT):
                    m3 = m3p.tile([128, CHW], b16, tag="m3", name=f"m3_{ch}_{ht}")
                    nc.vector.tensor_mul(
                        m3, e2t_sb[:, ht, csl], c2q_sb[:, ht, csl]
                    )
                    m3s.append(m3)
                for lc in range(CPT):
                    ct = ch * CPT + lc
                    tsl = slice(ct * 128, (ct + 1) * 128)
                    lsl = slice(ch * CHW + lc * 128, ch * CHW + (lc + 1) * 128)
                    ops = ps_out.tile([128, OUT], f32, tag="out", name=f"ops{ct}")
                    for ht in range(HT):
                        nc.tensor.matmul(
                            ops, e2t_sb[:, ht, tsl], wrt_sb[:, ht, :],
                            start=(ht == 0), stop=False,
                        )
                    for ht in range(HT):
                        nc.tensor.matmul(
                            ops, c2q_sb[:, ht, lsl], wrt_sb[:, 6 + ht, :],
                            start=False, stop=False,
                        )
                    for ht in range(HT):
                        nc.tensor.matmul(
                            ops, m3s[ht][:, lc * 128 : (lc + 1) * 128],
                            wrt_sb[:, 12 + ht, :],
                            start=False, stop=(ht == HT - 1),
                        )
                    nc.scalar.copy(out_sb[:, ct, :], ops)
                    emit_amr(3)

            amr_jobs = [
                (ht, ch2) for ch2 in range(NCH) for ht in range(HT)
            ]

            def emit_amr(n):
                for _ in range(n):
                    if not amr_jobs:
                        return
                    ht, ch2 = amr_jobs.pop(0)
                    csl2 = slice(ch2 * CHW, (ch2 + 1) * CHW)
                    m3 = m3p.tile(
                        [128, CHW], b16, tag="m3", name=f"am{ch2}_{ht}"
                    )
                    nc.vector.affine_mul_reduce(
                        out=m3,
                        accum_out=u_sb[:, ht, ch2 : ch2 + 1],
                        in0=e2t_sb[:, ht, csl2],
                        in1=ebc_sb[:, csl2],
                        scale=1.0,
                        bias=0.0,
                    )

            pass1(0)
            pass1(1)
            emit_amr(24)
            # q2c finalize: q2c = U / S, fold into wrT tail
            nc.vector.reduce_sum(
                out=s_sum, in_=s_parts, axis=mybir.AxisListType.X
            )
            nc.vector.reciprocal_approx_fast(out=rs_sum, in_=s_sum)
            rps = ps_out.tile([128, CHW], f32, tag="out")
            nc.tensor.matmul(
                rps[:, 0:1], ones_row_f, rs_sum, start=True, stop=True
            )
            nc.vector.tensor_copy(rs_col, rps[:, 0:1])
            nc.vector.reduce_sum(
                out=q2c_sb, in_=u_sb, axis=mybir.AxisListType.X
            )
            nc.vector.tensor_scalar_mul(q2c_sb, q2c_sb, rs_col)
            for ht in range(HT):
                nc.vector.tensor_scalar_mul(
                    wq4_sb[:, ht, :], wrt_sb[:, 18 + ht, :],
                    q2c_sb[:, ht : ht + 1],
                )
                nc.vector.tensor_add(
                    wsum_sb[:, ht, :], wq4_sb[:, ht, :], wrt_sb[:, ht, :]
                )

            def fused_pass(ch):
                csl = slice(ch * CHW, (ch + 1) * CHW)
                m3s = []
                for ht in range(HT):
                    m3 = m3p.tile([128, CHW], b16, tag="m3", name=f"m3f{ch}_{ht}")
                    nc.vector.tensor_mul(
                        m3, e2t_sb[:, ht, csl], c2q_sb[:, ht, csl]
                    )
                    m3s.append(m3)
                for lc in range(CPT):
                    ct = ch * CPT + lc
                    tsl = slice(ct * 128, (ct + 1) * 128)
                    ops = ps_out.tile([128, OUT], f32, tag="out", name=f"opf{ct}")
                    for ht in range(HT):
                        nc.tensor.matmul(
                            ops, e2t_sb[:, ht, tsl], wsum_sb[:, ht, :],
                            start=(ht == 0), stop=False,
                        )
                    for ht in range(HT):
                        nc.tensor.matmul(
                            ops, c2q_sb[:, ht, ch * CHW + lc * 128 :
                                        ch * CHW + (lc + 1) * 128],
                            wrt_sb[:, 6 + ht, :], start=False, stop=False,
                        )
                    for ht in range(HT):
                        nc.tensor.matmul(
                            ops, m3s[ht][:, lc * 128 : (lc + 1) * 128],
                            wrt_sb[:, 12 + ht, :], start=False, stop=False,
                        )
                    nc.tensor.matmul(
                        ops, ones_row_b, bred_sb, start=False, stop=True,
                    )
                    od = odp.tile([128, OUT], f32, tag="od", name=f"odf{ct}")
                    nc.vector.tensor_copy(od, ops)
                    nc.sync.dma_start(out=out_d[tsl, :], in_=od)

            fused_pass(2)
            pass2(0)
            pass2(1)
            fused_pass(3)

    nc.compile()
    return nc


def _get_nc():
    if "nc" not in _CACHE:
        _CACHE["nc"] = _build_bass()
    return _CACHE["nc"]


def _hl8(x):
    """fp8 e4m3 hi/lo pair of x (lo = unscaled residual)."""
    f8 = ml_dtypes.float8_e4m3
    hi = np.asarray(x, f8)
    lo = (np.asarray(x, np.float32) - hi.astype(np.float32)).astype(f8)
    return hi, lo


def _in_maps(emb1, emb2, w_c, b_c, w_q, b_q, w_cq, b_cq, w_red, b_red):
    # host-side sharding + layout only: batch split, transposes, casts
    emb1 = np.asarray(emb1, np.float32)
    emb2 = np.asarray(emb2, np.float32)
    wcq = np.asarray(w_cq, np.float32).reshape(HT, 128).T
    wc = np.asarray(w_c, np.float32).reshape(HT, 128).T
    wq = np.asarray(w_q, np.float32).reshape(HT, 128).T
    wpk = np.ascontiguousarray(np.concatenate([wcq, wc, wq], axis=1))
    wrt = np.ascontiguousarray(np.asarray(w_red, np.float32).T).astype(bf16)
    bred = np.asarray(b_red, np.float32).reshape(1, OUT).astype(bf16)
    maps = []
    for b in range(B):
        e1w = np.ascontiguousarray((emb1[b] * w_cq[None, :]).T) * 16.0
        e1wh, e1wl = _hl8(e1w)
        e2t = np.ascontiguousarray(emb2[b].T)
        e2th, e2tl = _hl8(e2t)
        maps.append(
            {
                "e1": emb1[b].astype(bf16),
                "e1t": np.ascontiguousarray(emb1[b].T).astype(bf16),
                "e2t": e2t.astype(bf16),
                "e1wh": e1wh,
                "e1wl": e1wl,
                "e2th": e2th,
                "e2tl": e2tl,
                "wrt": wrt,
                "wpk": wpk,
                "bred": bred,
            }
        )
    return maps


def run(inputs, trace=False):
    from concourse.bass_utils import run_bass_kernel_spmd

    nc = _get_nc()
    maps = _in_maps(**inputs)
    res = run_bass_kernel_spmd(nc, maps, list(range(B)), trace=trace)
    out = np.stack([res.results[b]["out"] for b in range(B)], axis=0)
    return out.astype(np.float32), res


def kernel(**inputs) -> np.ndarray:
    out, _ = run(inputs, trace=False)
    return out



# revision 13
# speedup vs baseline: 1.1697x; 1.1697x over previous
"""BiDAF attention-flow kernel for Trainium2 (Bass/Tile), SPMD over 8 cores.

Math (per batch element b, one NeuronCore each):
    s[c,j]  = sc[c] + sq[j] + sum_h e2[c,h]*wcq[h]*e1[j,h]
    a       = softmax_j(s);  c2q = a @ e1
    b_att   = softmax_c(max_j s);  q2c = b_att @ e2
    out     = [e2, c2q, e2*c2q, e2*q2c] @ w_red.T + b_red

Precision plan (fp8 e4m3 DoubleRow = 2x per k-tile on HW; only
both-operands-single-fp8 matmuls win, so):
    phase A (scores):      bf16 (softmax error amplification)
    P8 = fp8(P * 128/mall) (max-normalized, stays in e4m3 normal range)
    L8 = ones8 @ P8        (fp8 DR; self-normalizing: 1/L8 scales c2q)
    phase B (c2q):         fp8 DR, P8 (moving) x e18 (stationary)
    c2q8 = fp8(psum*bcr)   (DVE eviction mul)
    m38  = fp8(e2t*c2q8)   (DVE)
    reduction: R1 = e2@wsum64 bf16 (dominant block, error-critical),
               R2 = c2q8@W2*64 fp8 DR, R3 = m38@W3*64 fp8 DR,
               all in 64x-scaled PSUM, evicted with scale 1/64.
    q2c folded into wsum64 = 64*(W1 + diag(q2c) W4) on device.
    out bf16 (host upcasts). Predicted rel err ~1.5e-2 (gate 2e-2).

Schedule: A is chunk-outer so it starts on the first quarter of e2t;
macc max + partition_all_reduce(max) run per-chunk on GpSimd behind A;
P8/L8/bcr per chunk on DVE; B per-chunk fp8 DR interleaved after A;
sc (bf16 rank-1 trick) fills the A->B dependency gap. The reduction is
split: R23 (fp8, no q2c dependency) -> out_sb staging; R1+bias (needs
wsum) -> final DVE add + DMA. b_att path (escb/ebc/AMR/fold) threads
through Scalar/DVE between the eviction streams.
"""

import numpy as np
import ml_dtypes

B, Q, C, H, OUT = 8, 512, 2048, 768, 300
HT, JT, CT = H // 128, Q // 128, C // 128  # 6, 4, 16
NCH, CHW = 4, 512  # c chunks
CPT = 4  # c-tiles per chunk
PSCALE = 128.0  # P8 max-normalized scale (e4m3 max finite = 240)
WS = 64.0       # weight-side scale for the reduction layer

bf16 = ml_dtypes.bfloat16
f8_np = ml_dtypes.float8_e4m3

_CACHE = {}


def _build_bass():
    import concourse.tile as tile
    from concourse import mybir, bass_isa, library_config, bacc

    f32 = mybir.dt.float32
    b16 = mybir.dt.bfloat16
    f8 = mybir.dt.float8e4
    AF = mybir.ActivationFunctionType
    OP = mybir.AluOpType
    DR = mybir.MatmulPerfMode.DoubleRow

    nc = bacc.Bacc("TRN2", target_bir_lowering=False, debug=False)

    e1t_d = nc.dram_tensor("e1t", [H, Q], b16, kind="ExternalInput").ap()
    e18_d = nc.dram_tensor("e18", [Q, H], f8, kind="ExternalInput").ap()
    e2t_d = nc.dram_tensor("e2t", [H, C], b16, kind="ExternalInput").ap()
    wrt14_d = nc.dram_tensor("wrt14", [12 * 128, OUT], b16, kind="ExternalInput").ap()
    w2s_d = nc.dram_tensor("w2s", [H, OUT], f8, kind="ExternalInput").ap()
    w3s_d = nc.dram_tensor("w3s", [H, OUT], f8, kind="ExternalInput").ap()
    wpk_d = nc.dram_tensor("wpk", [128, 3 * HT], f32, kind="ExternalInput").ap()
    bred_d = nc.dram_tensor("bred", [1, OUT], b16, kind="ExternalInput").ap()
    out_d = nc.dram_tensor("out", [C, OUT], b16, kind="ExternalOutput").ap()

    with tile.TileContext(nc) as tc:
        with (
            tc.tile_pool(name="singles", bufs=1) as singles,
            tc.tile_pool(name="scr", bufs=4) as scrp,
            tc.tile_pool(name="odma", bufs=4) as odp,
            tc.tile_pool(name="ps_mm", bufs=4, space="PSUM") as ps_mm,
            tc.tile_pool(name="ps_out", bufs=4, space="PSUM") as ps_out,
        ):
            nc.gpsimd.load_library(library_config.attn)

            # ---- persistent SBUF tensors -------------------------------
            e1t_sb = singles.tile([128, HT, Q], b16)     # emb1.T, h on parts
            e1w_sb = singles.tile([128, HT, Q], b16)     # wcq * emb1.T
            e18_sb = singles.tile([128, JT, H], f8)      # emb1 fp8, j on parts
            e2t_sb = singles.tile([128, HT, C], b16)     # emb2.T, h on parts
            wrt14_sb = singles.tile([128, 12, OUT], b16)  # 64*[W1;W4] k-tiles
            w2s_sb = singles.tile([128, HT, OUT], f8)    # 64*W2.T fp8
            w3s_sb = singles.tile([128, HT, OUT], f8)    # 64*W3.T fp8
            wsum_sb = singles.tile([128, HT, OUT], b16)  # 64*(W1+q2c*W4)
            wpk_sb = singles.tile([128, 3 * HT], f32)
            wq_sb = singles.tile([128, HT], b16)
            wc_mat = singles.tile([128, HT, 128], b16)   # wc rank-1 bcast
            ones_mat = singles.tile([128, 128], b16)
            ones8 = singles.tile([128, 2, 128], f8)      # L8 DR stationary
            ones_row_b = singles.tile([1, 128], b16)
            ones_row_f = singles.tile([1, 128], f32)
            bred_sb = singles.tile([1, OUT], b16)
            sq_sb = singles.tile([128, JT], f32)
            pt_sb = singles.tile([128, NCH, JT, CHW], b16)  # P, ch-major
            p8_sb = singles.tile([128, NCH, JT, CHW], f8)   # 128*P/mall fp8
            macc = singles.tile([128, C], b16)
            mall = singles.tile([128, C], f32)
            rmall = singles.tile([128, C], f32)
            escb = singles.tile([128, C], b16)
            ebc = singles.tile([128, C], b16)
            bcr_sb = singles.tile([128, C], f32)         # 1/L8 bcast
            c2q8_sb = singles.tile([128, HT, C], f8)
            m38_sb = singles.tile([128, HT, C], f8)
            u_sb = singles.tile([128, HT], f32)          # q2c numerator
            s_sum = singles.tile([1, 1], f32)
            rs_sum = singles.tile([1, 1], f32)
            rs_col = singles.tile([128, 1], f32)
            q2c_sb = singles.tile([128, HT], f32)
            out_sb = singles.tile([128, CT, OUT], b16)   # (b2+b3)/64 staging

            # ---- loads (ordered for earliest PE start) -----------------
            nc.sync.dma_start(
                out=e1t_sb, in_=e1t_d.rearrange("(t p) j -> p t j", p=128)
            )
            nc.sync.dma_start(out=wpk_sb, in_=wpk_d)
            wcq_sb = wpk_sb[:, 0:HT]
            nc.vector.memset(ones_mat, 1.0)
            nc.vector.memset(ones8, 1.0)
            nc.vector.memset(ones_row_b, 1.0)
            nc.vector.memset(ones_row_f, 1.0)
            nc.vector.memset(macc, 0.0)
            nc.vector.tensor_copy(wq_sb, wpk_sb[:, 2 * HT : 3 * HT])
            for ht in range(HT):
                nc.vector.tensor_scalar_mul(
                    wc_mat[:, ht, :], ones_mat, wpk_sb[:, HT + ht : HT + ht + 1]
                )
            # HAM warm-up: keep the PE clocked while inputs stream in
            wps = ps_mm.tile([128, CHW], f32, tag="mm", name="warm")
            for _ in range(70):
                nc.tensor.matmul(wps[:, 0:128], ones_mat, ones_mat,
                                 start=True, stop=True)
            nc.vector.tensor_copy(rs_col, wps[:, 0:1])
            for ht in range(HT):
                nc.vector.tensor_scalar_mul(
                    e1w_sb[:, ht, :], e1t_sb[:, ht, :], wcq_sb[:, ht : ht + 1]
                )
            e2t_r = e2t_d.rearrange("(t p) c -> p t c", p=128)
            for ch in range(NCH):
                csl = slice(ch * CHW, (ch + 1) * CHW)
                for ht in range(HT):
                    nc.sync.dma_start(out=e2t_sb[:, ht, csl], in_=e2t_r[:, ht, csl])
            nc.sync.dma_start(
                out=e18_sb, in_=e18_d.rearrange("(t p) h -> p t h", p=128)
            )
            nc.sync.dma_start(
                out=wrt14_sb, in_=wrt14_d.rearrange("(t p) o -> p t o", p=128)
            )
            nc.sync.dma_start(
                out=w2s_sb, in_=w2s_d.rearrange("(t p) o -> p t o", p=128)
            )
            nc.sync.dma_start(
                out=w3s_sb, in_=w3s_d.rearrange("(t p) o -> p t o", p=128)
            )
            nc.sync.dma_start(out=bred_sb, in_=bred_d)

            # ---- sq columns (tiny, feeds exp bias) ---------------------
            for jt in range(JT):
                ps = ps_mm.tile([128, CHW], f32, tag="mm")
                for ht in range(HT):
                    nc.tensor.matmul(
                        ps[:, 0:1],
                        e1t_sb[:, ht, jt * 128 : (jt + 1) * 128],
                        wq_sb[:, ht : ht + 1],
                        start=(ht == 0),
                        stop=(ht == HT - 1),
                    )
                nc.vector.tensor_copy(sq_sb[:, jt : jt + 1], ps[:, 0:1])

            # ---- phase A: bf16 score matmuls, chunk-outer --------------
            # per (ch, jt): 6 ht matmuls -> exp -> pt; gpsimd maxes chain
            # into macc, then per-chunk partition_all_reduce(max) -> mall,
            # DVE: rmall = 128/mall, P8 = pt*rmall (fp8)
            def a_chunk(ch):
                csl = slice(ch * CHW, (ch + 1) * CHW)
                for jt in range(JT):
                    ps = ps_mm.tile([128, CHW], f32, tag="mm", name=f"a{ch}_{jt}")
                    for ht in range(HT):
                        nc.tensor.matmul(
                            ps,
                            e1w_sb[:, ht, jt * 128 : (jt + 1) * 128],
                            e2t_sb[:, ht, csl],
                            start=(ht == 0),
                            stop=(ht == HT - 1),
                        )
                    nc.scalar.activation(
                        out=pt_sb[:, ch, jt, :], in_=ps, func=AF.Exp,
                        bias=sq_sb[:, jt : jt + 1], scale=1.0,
                    )
                    nc.vector.tensor_max(
                        macc[:, csl], macc[:, csl], pt_sb[:, ch, jt, :]
                    )
                nc.gpsimd.partition_all_reduce(
                    mall[:, csl], macc[:, csl], channels=128,
                    reduce_op=bass_isa.ReduceOp.max,
                )
                nc.vector.reciprocal_approx_fast(
                    out=rmall[:, csl], in_=mall[:, csl]
                )
                for jt in range(JT):
                    nc.vector.scalar_tensor_tensor(
                        out=p8_sb[:, ch, jt, :],
                        in0=pt_sb[:, ch, jt, :],
                        scalar=float(PSCALE),
                        in1=rmall[:, csl],
                        op0=OP.mult,
                        op1=OP.mult,
                    )

            for ch in range(NCH):
                a_chunk(ch)

            # ---- per-chunk L8 (fp8 DR) + bcr, then B (fp8 DR) ----------
            def l8_chunk(ch):
                csl = slice(ch * CHW, (ch + 1) * CHW)
                lps = ps_mm.tile([128, CHW], f32, tag="mm", name=f"l8_{ch}")
                for jp in range(JT // 2):
                    nc.tensor.matmul(
                        lps, ones8,
                        p8_sb[:, ch, 2 * jp : 2 * jp + 2, :],
                        start=(jp == 0), stop=(jp == JT // 2 - 1),
                        perf_mode=DR,
                    )
                nc.vector.reciprocal_approx_fast(out=bcr_sb[:, csl], in_=lps)

            def b_chunk(ch):
                # c2qT[h, csl] += e18[jp-pair, h-tile].T @ P8[jp-pair, csl]
                csl = slice(ch * CHW, (ch + 1) * CHW)
                for hg in range(2):          # ht groups of 3 (PSUM budget)
                    cps = [
                        ps_mm.tile([128, CHW], f32, tag="mm", name=f"b{ch}_{hg}_{i}")
                        for i in range(3)
                    ]
                    for i in range(3):
                        ht = hg * 3 + i
                        for jp in range(JT // 2):
                            nc.tensor.matmul(
                                cps[i],
                                e18_sb[:, 2 * jp : 2 * jp + 2,
                                       ht * 128 : (ht + 1) * 128],
                                p8_sb[:, ch, 2 * jp : 2 * jp + 2, :],
                                start=(jp == 0), stop=(jp == JT // 2 - 1),
                                perf_mode=DR,
                            )
                    for i in range(3):
                        ht = hg * 3 + i
                        nc.vector.tensor_mul(
                            c2q8_sb[:, ht, csl], cps[i], bcr_sb[:, csl]
                        )
                        nc.vector.tensor_mul(
                            m38_sb[:, ht, csl], e2t_sb[:, ht, csl],
                            c2q8_sb[:, ht, csl],
                        )

            # ---- exp(sc) rank-1 trick (bf16), feeds b_att --------------
            def sc_chunks(chs):
                for ch in chs:
                    csl = slice(ch * CHW, (ch + 1) * CHW)
                    ps = ps_mm.tile([128, CHW], f32, tag="mm", name=f"sc{ch}")
                    for ht in range(HT):
                        nc.tensor.matmul(
                            ps, wc_mat[:, ht, :], e2t_sb[:, ht, csl],
                            start=(ht == 0), stop=(ht == HT - 1),
                        )
                    nc.scalar.activation(
                        out=escb[:, csl], in_=ps, func=AF.Exp,
                        bias=0.0, scale=1.0,
                    )

            l8_chunk(0)
            b_chunk(0)
            l8_chunk(1)
            b_chunk(1)
            sc_chunks([0, 1, 2, 3])
            l8_chunk(2)
            b_chunk(2)
            l8_chunk(3)
            b_chunk(3)

            # ---- b_att numerator: E = mall*escb; u = sum_c e2t*E -------
            nc.vector.tensor_mul(ebc, mall, escb)
            amr_scr = [
                scrp.tile([128, C], b16, tag="scr", name=f"amr{i}")
                for i in range(2)
            ]
            for ht in range(HT):
                nc.vector.affine_mul_reduce(
                    out=amr_scr[ht % 2],
                    accum_out=u_sb[:, ht : ht + 1],
                    in0=e2t_sb[:, ht, :],
                    in1=ebc,
                    scale=1.0,
                    bias=0.0,
                )
            nc.vector.reduce_sum(
                out=s_sum, in_=ebc[0:1, :], axis=mybir.AxisListType.X
            )
            nc.vector.reciprocal_approx_fast(out=rs_sum, in_=s_sum)
            rps = ps_out.tile([128, OUT], f32, tag="out", name="rs")
            nc.tensor.matmul(
                rps[:, 0:1], ones_row_f, rs_sum, start=True, stop=True
            )
            nc.vector.tensor_copy(rs_col, rps[:, 0:1])
            nc.vector.tensor_scalar_mul(q2c_sb, u_sb, rs_col)
            # wsum64 = 64*W1 + q2c * 64*W4
            for ht in range(HT):
                nc.vector.tensor_scalar_mul(
                    wsum_sb[:, ht, :], wrt14_sb[:, 6 + ht, :],
                    q2c_sb[:, ht : ht + 1],
                )
                nc.vector.tensor_add(
                    wsum_sb[:, ht, :], wsum_sb[:, ht, :], wrt14_sb[:, ht, :]
                )

            # ---- reduction: R23 (fp8 DR) -> out_sb; R1+bias -> add -----
            def r23(ct):
                ch = ct // CPT
                tsl = slice(ct * 128, (ct + 1) * 128)
                ops = ps_out.tile([128, OUT], f32, tag="out", name=f"r23_{ct}")
                for hp in range(HT // 2):
                    nc.tensor.matmul(
                        ops,
                        c2q8_sb[:, 2 * hp : 2 * hp + 2, tsl],
                        w2s_sb[:, 2 * hp : 2 * hp + 2, :],
                        start=(hp == 0), stop=False,
                        perf_mode=DR,
                    )
                for hp in range(HT // 2):
                    nc.tensor.matmul(
                        ops,
                        m38_sb[:, 2 * hp : 2 * hp + 2, tsl],
                        w3s_sb[:, 2 * hp : 2 * hp + 2, :],
                        start=False, stop=(hp == HT // 2 - 1),
                        perf_mode=DR,
                    )
                nc.scalar.activation(
                    out=out_sb[:, ct, :], in_=ops, func=AF.Copy,
                    bias=0.0, scale=1.0 / WS,
                )

            def r1(ct):
                tsl = slice(ct * 128, (ct + 1) * 128)
                ops = ps_out.tile([128, OUT], f32, tag="out", name=f"r1_{ct}")
                for ht in range(HT):
                    nc.tensor.matmul(
                        ops, e2t_sb[:, ht, tsl], wsum_sb[:, ht, :],
                        start=(ht == 0), stop=False,
                    )
                nc.tensor.matmul(
                    ops, ones_row_b, bred_sb, start=False, stop=True,
                )
                od = odp.tile([128, OUT], b16, tag="od", name=f"od{ct}")
                nc.vector.scalar_tensor_tensor(
                    out=od, in0=ops, scalar=1.0 / WS, in1=out_sb[:, ct, :],
                    op0=OP.mult, op1=OP.add,
                )
                nc.sync.dma_start(out=out_d[tsl, :], in_=od)

            for ct in range(3):
                r23(ct)
            for ct in range(CT):
                r1(ct)
                if ct + 3 < CT:
                    r23(ct + 3)

    nc.compile()
    return nc


def _get_nc():
    if "nc" not in _CACHE:
        _CACHE["nc"] = _build_bass()
    return _CACHE["nc"]


def _in_maps(emb1, emb2, w_c, b_c, w_q, b_q, w_cq, b_cq, w_red, b_red):
    # host-side sharding + layout only: batch split, transposes, casts
    emb1 = np.asarray(emb1, np.float32)
    emb2 = np.asarray(emb2, np.float32)
    wcq = np.asarray(w_cq, np.float32).reshape(HT, 128).T
    wc = np.asarray(w_c, np.float32).reshape(HT, 128).T
    wq = np.asarray(w_q, np.float32).reshape(HT, 128).T
    wpk = np.ascontiguousarray(np.concatenate([wcq, wc, wq], axis=1))
    wrt = np.ascontiguousarray(np.asarray(w_red, np.float32).T)  # (4H, OUT)
    wrt14 = np.concatenate([wrt[0:H] * WS, wrt[3 * H : 4 * H] * WS], axis=0)
    w2s = (wrt[H : 2 * H] * WS).astype(f8_np)
    w3s = (wrt[2 * H : 3 * H] * WS).astype(f8_np)
    bred = (np.asarray(b_red, np.float32).reshape(1, OUT) * WS).astype(bf16)
    maps = []
    for b in range(B):
        maps.append(
            {
                "e1t": np.ascontiguousarray(emb1[b].T).astype(bf16),
                "e18": emb1[b].astype(f8_np),
                "e2t": np.ascontiguousarray(emb2[b].T).astype(bf16),
                "wrt14": wrt14.astype(bf16),
                "w2s": w2s,
                "w3s": w3s,
                "wpk": wpk,
                "bred": bred,
            }
        )
    return maps


def run(inputs, trace=False):
    from concourse.bass_utils import run_bass_kernel_spmd

    nc = _get_nc()
    maps = _in_maps(**inputs)
    res = run_bass_kernel_spmd(nc, maps, list(range(B)), trace=trace)
    out = np.stack(
        [res.results[b]["out"].astype(np.float32) for b in range(B)], axis=0
    )
    return out, res


def kernel(**inputs) -> np.ndarray:
    out, _ = run(inputs, trace=False)
    return out


# revision 24
# speedup vs baseline: 1.3365x; 1.1427x over previous
"""BiDAF attention-flow kernel for Trainium2 (Bass/Tile), SPMD over 8 cores.

Math (per batch element b, one NeuronCore each):
    s[c,j]  = sc[c] + sq[j] + sum_h e2[c,h]*wcq[h]*e1[j,h]
    a       = softmax_j(s);  c2q = a @ e1
    b_att   = softmax_c(max_j s);  q2c = b_att @ e2
    out     = [e2, c2q, e2*c2q, e2*q2c] @ w_red.T + b_red

Precision plan (fp8 e4m3 DoubleRow = 2x per k-tile on TRN2 HW; only
both-operands-single-fp8 matmuls win):
    phase A (scores):  bf16 (softmax amplifies score errors)
    P8 = fp8(128 * P / L)   L = ones @ P (bf16 PE matmul, per chunk);
                            prenorm means c2q eviction is a constant
                            1/128 scale -> plain Scalar-engine copies
    phase B (c2q):     fp8 DR, P8 (moving) x e18 (stationary)
    c2q8 = fp8(psum/128)    (Scalar activation Copy, scale=1/128)
    m38  = fp8(e2t*c2q8)    (DVE, [128,1024] chunk-pair ops)
    sc   = fp8 DR rank-1    (errors dilute through diffuse b_att)
    q2c  = PE matvecs: E-col (DMA-transposed ebc) x c-major e2
    reduction per c-tile: R2 = c2q8@64W2 fp8 DR + R3 = m38@64W3 fp8 DR
        + R1 = e2@wsum64 bf16 + bias, one PSUM group, Scalar evict /64
    wsum64 = 64*(W1 + diag(q2c) W4) folded on DVE (scalar_tensor_tensor)
    out bf16 (host upcasts). Predicted rel err ~1.67e-2 (gate 2e-2).
"""

import numpy as np
import ml_dtypes

B, Q, C, H, OUT = 8, 512, 2048, 768, 300
HT, JT, CT = H // 128, Q // 128, C // 128  # 6, 4, 16
NCH, CHW = 4, 512  # c chunks
PSCALE = 128.0  # P8 prenorm scale (e4m3 max finite = 240)
WS = 64.0       # weight-side scale for the reduction layer
SCS = 16.0      # wc scale for the fp8 sc matmuls

bf16 = ml_dtypes.bfloat16
f8_np = ml_dtypes.float8_e4m3

_CACHE = {}


def _build_bass():
    import concourse.tile as tile
    from concourse import mybir, bass_isa, library_config, bacc

    f32 = mybir.dt.float32
    b16 = mybir.dt.bfloat16
    f8 = mybir.dt.float8e4
    AF = mybir.ActivationFunctionType
    OP = mybir.AluOpType
    DR = mybir.MatmulPerfMode.DoubleRow

    nc = bacc.Bacc("TRN2", target_bir_lowering=False, debug=False)

    e1t_d = nc.dram_tensor("e1t", [H, Q], b16, kind="ExternalInput").ap()
    e18_d = nc.dram_tensor("e18", [Q, H], f8, kind="ExternalInput").ap()
    e2t_d = nc.dram_tensor("e2t", [H, C], b16, kind="ExternalInput").ap()
    e2t8_d = nc.dram_tensor("e2t8", [H, C], f8, kind="ExternalInput").ap()
    e2cm_d = nc.dram_tensor("e2cm", [C, H], b16, kind="ExternalInput").ap()
    wrt14_d = nc.dram_tensor("wrt14", [12 * 128, OUT], b16, kind="ExternalInput").ap()
    w2s_d = nc.dram_tensor("w2s", [H, OUT], f8, kind="ExternalInput").ap()
    w3s_d = nc.dram_tensor("w3s", [H, OUT], f8, kind="ExternalInput").ap()
    wpk_d = nc.dram_tensor("wpk", [128, 3 * HT], f32, kind="ExternalInput").ap()
    bred_d = nc.dram_tensor("bred", [1, OUT], b16, kind="ExternalInput").ap()
    out_d = nc.dram_tensor("out", [C, OUT], b16, kind="ExternalOutput").ap()
    ebcrow_d = nc.dram_tensor("ebcrow", [1, C], b16, kind="Internal").ap()
    urow_d = nc.dram_tensor("urow", [1, H], f32, kind="Internal").ap()

    with tile.TileContext(nc) as tc:
        with (
            tc.tile_pool(name="singles", bufs=1) as singles,
            tc.tile_pool(name="odma", bufs=4) as odp,
            tc.tile_pool(name="ps_mm", bufs=4, space="PSUM") as ps_mm,
            tc.tile_pool(name="ps_out", bufs=4, space="PSUM") as ps_out,
        ):
            nc.gpsimd.load_library(library_config.attn)

            # ---- persistent SBUF tensors -------------------------------
            e1t_sb = singles.tile([128, HT, Q], b16)
            e1w_sb = singles.tile([128, HT, Q], b16)     # wcq * emb1.T
            e18_sb = singles.tile([128, JT, H], f8)      # emb1 fp8, j parts
            e2t_sb = singles.tile([128, HT, C], b16)
            e2t8_sb = singles.tile([128, HT, C], f8)
            e2cm_sb = singles.tile([128, CT, H], b16)    # emb2 c-major
            wrt14_sb = singles.tile([128, 12, OUT], b16)  # 64*[W1;W4]
            w2s_sb = singles.tile([128, HT, OUT], f8)
            w3s_sb = singles.tile([128, HT, OUT], f8)
            wsum_sb = singles.tile([128, HT, OUT], b16)  # 64*(W1+q2c*W4)
            wpk_sb = singles.tile([128, 3 * HT], f32)
            wq_sb = singles.tile([128, HT], b16)
            wcm8 = singles.tile([128, HT, 128], f8)      # 16*wc rank-1 fp8
            ones_mat = singles.tile([128, 128], b16)
            ones_col = singles.tile([128, 1], b16)
            ones_row_b = singles.tile([1, 128], b16)
            ones_row_f = singles.tile([1, 128], f32)
            bred_sb = singles.tile([1, OUT], b16)
            sq_sb = singles.tile([128, JT], f32)
            pt_sb = singles.tile([128, NCH, JT, CHW], b16)
            p8_sb = singles.tile([128, NCH, JT, CHW], f8)
            macc = singles.tile([128, C], b16)
            mall = singles.tile([128, C], b16)
            rl_sb = singles.tile([128, C], f32)          # 1/L
            escb = singles.tile([128, C], b16)
            ebc = singles.tile([128, C], b16)            # E = mall*escb
            ecol = singles.tile([128, CT], b16)          # E, c on parts
            c2q8_sb = singles.tile([128, HT, C], f8)
            m38_sb = singles.tile([128, HT, C], f8)
            urow = singles.tile([1, H], f32)
            rs_sum = singles.tile([1, 1], f32)
            rs_col = singles.tile([128, 1], f32)
            q2cT = singles.tile([128, HT], f32)          # q2c, h on parts
            q2cs = singles.tile([128, HT], f32)

            # ---- loads (ordered for earliest PE start) -----------------
            nc.sync.dma_start(
                out=e1t_sb, in_=e1t_d.rearrange("(t p) j -> p t j", p=128)
            )
            nc.sync.dma_start(out=wpk_sb, in_=wpk_d)
            wcq_sb = wpk_sb[:, 0:HT]
            nc.vector.memset(ones_mat, 1.0)
            nc.vector.memset(ones_col, 1.0)
            nc.vector.memset(ones_row_b, 1.0)
            nc.vector.memset(ones_row_f, 1.0)
            nc.vector.memset(macc, 0.0)
            nc.vector.tensor_copy(wq_sb, wpk_sb[:, 2 * HT : 3 * HT])
            # HAM warm-up: keep the PE clocked while inputs stream in
            wps = ps_mm.tile([128, CHW], f32, tag="mm", name="warm")
            for _ in range(55):
                nc.tensor.matmul(wps[:, 0:128], ones_mat, ones_mat,
                                 start=True, stop=True)
            nc.vector.tensor_copy(rs_col, wps[:, 0:1])
            for ht in range(HT):
                nc.vector.tensor_scalar_mul(
                    e1w_sb[:, ht, :], e1t_sb[:, ht, :], wcq_sb[:, ht : ht + 1]
                )
            for ht in range(HT):
                # 16*wc broadcast as fp8 rank-1 stationary (host pre-scales)
                nc.vector.tensor_scalar_mul(
                    wcm8[:, ht, :], ones_mat, wpk_sb[:, HT + ht : HT + ht + 1]
                )
            e2t_r = e2t_d.rearrange("(t p) c -> p t c", p=128)
            for ch in range(NCH):
                csl = slice(ch * CHW, (ch + 1) * CHW)
                for ht in range(HT):
                    nc.sync.dma_start(out=e2t_sb[:, ht, csl], in_=e2t_r[:, ht, csl])
            nc.sync.dma_start(
                out=e18_sb, in_=e18_d.rearrange("(t p) h -> p t h", p=128)
            )
            nc.sync.dma_start(
                out=e2t8_sb, in_=e2t8_d.rearrange("(t p) c -> p t c", p=128)
            )
            nc.sync.dma_start(
                out=wrt14_sb, in_=wrt14_d.rearrange("(t p) o -> p t o", p=128)
            )
            nc.sync.dma_start(
                out=w2s_sb, in_=w2s_d.rearrange("(t p) o -> p t o", p=128)
            )
            nc.sync.dma_start(
                out=w3s_sb, in_=w3s_d.rearrange("(t p) o -> p t o", p=128)
            )
            nc.sync.dma_start(
                out=e2cm_sb, in_=e2cm_d.rearrange("(t p) h -> p t h", p=128)
            )
            nc.sync.dma_start(out=bred_sb, in_=bred_d)

            # ---- sq columns (tiny, feeds exp bias) ---------------------
            for jt in range(JT):
                ps = ps_mm.tile([128, CHW], f32, tag="mm")
                for ht in range(HT):
                    nc.tensor.matmul(
                        ps[:, 0:1],
                        e1t_sb[:, ht, jt * 128 : (jt + 1) * 128],
                        wq_sb[:, ht : ht + 1],
                        start=(ht == 0),
                        stop=(ht == HT - 1),
                    )
                nc.vector.tensor_copy(sq_sb[:, jt : jt + 1], ps[:, 0:1])

            # ---- phase A: bf16 scores, chunk-outer, L + P8 per chunk ---
            def a_chunk(ch):
                csl = slice(ch * CHW, (ch + 1) * CHW)
                for jt in range(JT):
                    ps = ps_mm.tile([128, CHW], f32, tag="mm", name=f"a{ch}_{jt}")
                    for ht in range(HT):
                        nc.tensor.matmul(
                            ps,
                            e1w_sb[:, ht, jt * 128 : (jt + 1) * 128],
                            e2t_sb[:, ht, csl],
                            start=(ht == 0),
                            stop=(ht == HT - 1),
                        )
                    nc.scalar.activation(
                        out=pt_sb[:, ch, jt, :], in_=ps, func=AF.Exp,
                        bias=sq_sb[:, jt : jt + 1], scale=1.0,
                    )
                    nc.vector.tensor_max(
                        macc[:, csl], macc[:, csl], pt_sb[:, ch, jt, :]
                    )
                nc.gpsimd.partition_all_reduce(
                    mall[:, csl], macc[:, csl], channels=128,
                    reduce_op=bass_isa.ReduceOp.max,
                )

            def l_chunk(ch):
                csl = slice(ch * CHW, (ch + 1) * CHW)
                lps = ps_mm.tile([128, CHW], f32, tag="mm", name=f"l{ch}")
                for jt in range(JT):
                    nc.tensor.matmul(
                        lps, ones_mat, pt_sb[:, ch, jt, :],
                        start=(jt == 0), stop=(jt == JT - 1),
                    )
                nc.vector.reciprocal_approx_fast(out=rl_sb[:, csl], in_=lps)
                for jt in range(JT):
                    nc.vector.scalar_tensor_tensor(
                        out=p8_sb[:, ch, jt, :],
                        in0=pt_sb[:, ch, jt, :],
                        scalar=float(PSCALE),
                        in1=rl_sb[:, csl],
                        op0=OP.mult, op1=OP.mult,
                    )

            def b_chunk(ch):
                # c2qT[h, csl] = sum_j e18[j,h] * P8[j,csl]; evict = scalar
                # copy with constant 1/PSCALE scale
                csl = slice(ch * CHW, (ch + 1) * CHW)
                for hg in range(2):
                    cps = [
                        ps_mm.tile([128, CHW], f32, tag="mm",
                                   name=f"b{ch}_{hg}_{i}")
                        for i in range(3)
                    ]
                    for i in range(3):
                        ht = hg * 3 + i
                        for jp in range(JT // 2):
                            nc.tensor.matmul(
                                cps[i],
                                e18_sb[:, 2 * jp : 2 * jp + 2,
                                       ht * 128 : (ht + 1) * 128],
                                p8_sb[:, ch, 2 * jp : 2 * jp + 2, :],
                                start=(jp == 0), stop=(jp == JT // 2 - 1),
                                perf_mode=DR,
                            )
                    for i in range(3):
                        ht = hg * 3 + i
                        nc.scalar.activation(
                            out=c2q8_sb[:, ht, csl], in_=cps[i],
                            func=AF.Copy, bias=0.0, scale=1.0 / PSCALE,
                        )

            def sc_chunk(ch):
                # fp8 DR rank-1: psum = 16*sc broadcast on all partitions
                csl = slice(ch * CHW, (ch + 1) * CHW)
                ps = ps_mm.tile([128, CHW], f32, tag="mm", name=f"sc{ch}")
                for hp in range(HT // 2):
                    nc.tensor.matmul(
                        ps,
                        wcm8[:, 2 * hp : 2 * hp + 2, :],
                        e2t8_sb[:, 2 * hp : 2 * hp + 2, csl],
                        start=(hp == 0), stop=(hp == HT // 2 - 1),
                        perf_mode=DR,
                    )
                nc.scalar.activation(
                    out=escb[:, csl], in_=ps, func=AF.Exp,
                    bias=0.0, scale=1.0 / SCS,
                )

            a_chunk(0)
            l_chunk(0)
            a_chunk(1)
            l_chunk(1)
            b_chunk(0)
            a_chunk(2)
            l_chunk(2)
            b_chunk(1)
            a_chunk(3)
            l_chunk(3)
            sc_chunk(0)
            sc_chunk(1)
            b_chunk(2)
            sc_chunk(2)
            sc_chunk(3)
            b_chunk(3)

            # ---- m38 = e2t * c2q8, chunk-pair wide DVE ops -------------
            def m38_pair(half):
                fsl = slice(half * 2 * CHW, (half + 1) * 2 * CHW)
                for ht in range(HT):
                    nc.vector.tensor_mul(
                        m38_sb[:, ht, fsl], e2t_sb[:, ht, fsl],
                        c2q8_sb[:, ht, fsl],
                    )

            m38_pair(0)

            # ---- b_att: E row, transpose to parts, q2c on PE -----------
            nc.vector.tensor_mul(ebc, mall, escb)
            # E with c on partitions (transpose via DRAM bounce)
            nc.sync.dma_start(out=ebcrow_d, in_=ebc[0:1, :])
            nc.sync.dma_start(
                out=ecol, in_=ebcrow_d.rearrange("1 (t p) -> p t", p=128)
            )
            m38_pair(1)
            ups1 = ps_mm.tile([128, CHW], f32, tag="mm", name="u1")
            ups2 = ps_mm.tile([128, CHW], f32, tag="mm", name="u2")
            sps = ps_mm.tile([128, CHW], f32, tag="mm", name="sS")
            for ct in range(CT):
                nc.tensor.matmul(
                    ups1[0:1, 0:CHW], ecol[:, ct : ct + 1],
                    e2cm_sb[:, ct, 0:CHW],
                    start=(ct == 0), stop=(ct == CT - 1),
                )
                nc.tensor.matmul(
                    ups2[0:1, 0 : H - CHW], ecol[:, ct : ct + 1],
                    e2cm_sb[:, ct, CHW:H],
                    start=(ct == 0), stop=(ct == CT - 1),
                )
                nc.tensor.matmul(
                    sps[0:1, 0:1], ecol[:, ct : ct + 1], ones_col,
                    start=(ct == 0), stop=(ct == CT - 1),
                )
            nc.vector.tensor_copy(urow[:, 0:CHW], ups1[0:1, 0:CHW])
            nc.vector.tensor_copy(urow[:, CHW:H], ups2[0:1, 0 : H - CHW])
            nc.vector.reciprocal_approx_fast(out=rs_sum, in_=sps[0:1, 0:1])
            nc.sync.dma_start(out=urow_d, in_=urow)
            nc.sync.dma_start(
                out=q2cT, in_=urow_d.rearrange("1 (t p) -> p t", p=128)
            )
            rps = ps_mm.tile([128, CHW], f32, tag="mm", name="rsb")
            nc.tensor.matmul(
                rps[:, 0:1], ones_row_f, rs_sum, start=True, stop=True
            )
            nc.vector.tensor_copy(rs_col, rps[:, 0:1])
            nc.vector.tensor_scalar_mul(q2cs, q2cT, rs_col)
            # wsum64 = 64*W1 + q2c * 64*W4, one fused DVE op per ht
            for ht in range(HT):
                nc.vector.scalar_tensor_tensor(
                    out=wsum_sb[:, ht, :],
                    in0=wrt14_sb[:, 6 + ht, :],
                    scalar=q2cs[:, ht : ht + 1],
                    in1=wrt14_sb[:, ht, :],
                    op0=OP.mult, op1=OP.add,
                )

            # ---- reduction: R23 (fp8 DR) + R1 (bf16) + bias, fused -----
            def r23(ct):
                tsl = slice(ct * 128, (ct + 1) * 128)
                ops = ps_out.tile([128, OUT], f32, tag="out", name=f"o{ct}")
                for hp in range(HT // 2):
                    nc.tensor.matmul(
                        ops,
                        c2q8_sb[:, 2 * hp : 2 * hp + 2, tsl],
                        w2s_sb[:, 2 * hp : 2 * hp + 2, :],
                        start=(hp == 0), stop=False,
                        perf_mode=DR,
                    )
                for hp in range(HT // 2):
                    nc.tensor.matmul(
                        ops,
                        m38_sb[:, 2 * hp : 2 * hp + 2, tsl],
                        w3s_sb[:, 2 * hp : 2 * hp + 2, :],
                        start=False, stop=False,
                        perf_mode=DR,
                    )
                return ops

            def r1(ct, ops):
                tsl = slice(ct * 128, (ct + 1) * 128)
                for ht in range(HT):
                    nc.tensor.matmul(
                        ops, e2t_sb[:, ht, tsl], wsum_sb[:, ht, :],
                        start=False, stop=False,
                    )
                nc.tensor.matmul(
                    ops, ones_row_b, bred_sb, start=False, stop=True,
                )
                od = odp.tile([128, OUT], b16, tag="od", name=f"od{ct}")
                nc.scalar.activation(
                    out=od, in_=ops, func=AF.Copy, bias=0.0, scale=1.0 / WS,
                )
                nc.sync.dma_start(out=out_d[tsl, :], in_=od)

            WINDOW = 3
            open_ps = {}
            for ct in range(WINDOW):
                open_ps[ct] = r23(ct)
            for ct in range(CT):
                r1(ct, open_ps.pop(ct))
                nxt = ct + WINDOW
                if nxt < CT:
                    open_ps[nxt] = r23(nxt)

    nc.compile()
    return nc


def _get_nc():
    if "nc" not in _CACHE:
        _CACHE["nc"] = _build_bass()
    return _CACHE["nc"]


def _in_maps(emb1, emb2, w_c, b_c, w_q, b_q, w_cq, b_cq, w_red, b_red):
    # host-side sharding + layout only: batch split, transposes, casts
    emb1 = np.asarray(emb1, np.float32)
    emb2 = np.asarray(emb2, np.float32)
    wcq = np.asarray(w_cq, np.float32).reshape(HT, 128).T
    wc = np.asarray(w_c, np.float32).reshape(HT, 128).T * SCS
    wq = np.asarray(w_q, np.float32).reshape(HT, 128).T
    wpk = np.ascontiguousarray(np.concatenate([wcq, wc, wq], axis=1))
    wrt = np.ascontiguousarray(np.asarray(w_red, np.float32).T)  # (4H, OUT)
    wrt14 = np.concatenate([wrt[0:H] * WS, wrt[3 * H : 4 * H] * WS], axis=0)
    w2s = (wrt[H : 2 * H] * WS).astype(f8_np)
    w3s = (wrt[2 * H : 3 * H] * WS).astype(f8_np)
    bred = (np.asarray(b_red, np.float32).reshape(1, OUT) * WS).astype(bf16)
    maps = []
    for b in range(B):
        e2t = np.ascontiguousarray(emb2[b].T)
        maps.append(
            {
                "e1t": np.ascontiguousarray(emb1[b].T).astype(bf16),
                "e18": emb1[b].astype(f8_np),
                "e2t": e2t.astype(bf16),
                "e2t8": e2t.astype(f8_np),
                "e2cm": emb2[b].astype(bf16),
                "wrt14": wrt14.astype(bf16),
                "w2s": w2s,
                "w3s": w3s,
                "wpk": wpk,
                "bred": bred,
            }
        )
    return maps


def run(inputs, trace=False):
    from concourse.bass_utils import run_bass_kernel_spmd

    nc = _get_nc()
    maps = _in_maps(**inputs)
    res = run_bass_kernel_spmd(nc, maps, list(range(B)), trace=trace)
    out = np.stack(
        [res.results[b]["out"].astype(np.float32) for b in range(B)], axis=0
    )
    return out, res


def kernel(**inputs) -> np.ndarray:
    out, _ = run(inputs, trace=False)
    return out


# revision 32
# speedup vs baseline: 1.5514x; 1.1608x over previous
"""BiDAF attention-flow kernel for Trainium2 (Bass/Tile), SPMD over 8 cores.

Math (per batch element b, one NeuronCore each):
    s[c,j]  = sc[c] + sq[j] + sum_h e2[c,h]*wcq[h]*e1[j,h]
    a       = softmax_j(s);  c2q = a @ e1
    b_att   = softmax_c(max_j s);  q2c = b_att @ e2
    out     = [e2, c2q, e2*c2q, e2*q2c] @ w_red.T + b_red

Precision plan (fp8 e4m3 DoubleRow = 2x per k-tile on TRN2 HW; only
both-operands-single-fp8 matmuls win):
    phase A (scores):  bf16 (softmax amplifies score errors)
    P8 = fp8(128 * P / L)   L = ones @ P (bf16 PE matmul, per chunk);
                            prenorm means c2q eviction is a constant
                            1/128 scale -> plain Scalar-engine copies
    phase B (c2q):     fp8 DR, P8 (moving) x e18 (stationary)
    c2q8 = fp8(psum/128)    (Scalar activation Copy, scale=1/128)
    m38  = fp8(e2t*c2q8)    (DVE, [128,1024] chunk-pair ops)
    sc   = fp8 DR rank-1    (errors dilute through diffuse b_att)
    q2c  = PE matvecs: E-col (DMA-transposed ebc) x c-major e2
    reduction per c-tile: R2 = c2q8@64W2 fp8 DR + R3 = m38@64W3 fp8 DR
        + R1 = e2@wsum64 bf16 + bias, one PSUM group, Scalar evict /64
    wsum64 = 64*(W1 + diag(q2c) W4) folded on DVE (scalar_tensor_tensor)
    out bf16 (host upcasts). Predicted rel err ~1.67e-2 (gate 2e-2).
"""

import numpy as np
import ml_dtypes

B, Q, C, H, OUT = 8, 512, 2048, 768, 300
HT, JT, CT = H // 128, Q // 128, C // 128  # 6, 4, 16
NCH, CHW = 4, 512  # c chunks
PSCALE = 128.0  # P8 prenorm scale (e4m3 max finite = 240)
WS = 64.0       # weight-side scale for the reduction layer
SCS = 16.0      # wc scale for the fp8 sc matmuls

bf16 = ml_dtypes.bfloat16
f8_np = ml_dtypes.float8_e4m3

_CACHE = {}


def _build_bass():
    import concourse.tile as tile
    from concourse import mybir, bass_isa, library_config, bacc

    f32 = mybir.dt.float32
    b16 = mybir.dt.bfloat16
    f8 = mybir.dt.float8e4
    AF = mybir.ActivationFunctionType
    OP = mybir.AluOpType
    DR = mybir.MatmulPerfMode.DoubleRow

    nc = bacc.Bacc("TRN2", target_bir_lowering=False, debug=False)

    e1t_d = nc.dram_tensor("e1t", [H, Q], b16, kind="ExternalInput").ap()
    e18_d = nc.dram_tensor("e18", [Q, H], f8, kind="ExternalInput").ap()
    e2t_d = nc.dram_tensor("e2t", [H, C], b16, kind="ExternalInput").ap()
    e2t8_d = nc.dram_tensor("e2t8", [H, C], f8, kind="ExternalInput").ap()
    e2cm_d = nc.dram_tensor("e2cm", [C, H], b16, kind="ExternalInput").ap()
    wrt14_d = nc.dram_tensor("wrt14", [12 * 128, OUT], b16, kind="ExternalInput").ap()
    w2s_d = nc.dram_tensor("w2s", [H, OUT], f8, kind="ExternalInput").ap()
    w3s_d = nc.dram_tensor("w3s", [H, OUT], f8, kind="ExternalInput").ap()
    wpk_d = nc.dram_tensor("wpk", [128, 3 * HT], f32, kind="ExternalInput").ap()
    bred_d = nc.dram_tensor("bred", [1, OUT], b16, kind="ExternalInput").ap()
    out_d = nc.dram_tensor("out", [C, OUT], b16, kind="ExternalOutput").ap()
    ebcrow_d = nc.dram_tensor("ebcrow", [1, C], b16, kind="Internal").ap()
    urow_d = nc.dram_tensor("urow", [1, H], f32, kind="Internal").ap()

    with tile.TileContext(nc) as tc:
        with (
            tc.tile_pool(name="singles", bufs=1) as singles,
            tc.tile_pool(name="odma", bufs=4) as odp,
            tc.tile_pool(name="ps_mm", bufs=4, space="PSUM") as ps_mm,
            tc.tile_pool(name="ps_out", bufs=4, space="PSUM") as ps_out,
        ):
            nc.gpsimd.load_library(library_config.attn)

            # ---- persistent SBUF tensors -------------------------------
            e1t_sb = singles.tile([128, HT, Q], b16)
            e1w_sb = singles.tile([128, HT, Q], b16)     # wcq * emb1.T
            e18_sb = singles.tile([128, JT, H], f8)      # emb1 fp8, j parts
            e2t_sb = singles.tile([128, HT, C], b16)
            e2t8_sb = singles.tile([128, HT, C], f8)
            e2cm_sb = singles.tile([128, CT, H], b16)    # emb2 c-major
            wrt14_sb = singles.tile([128, 12, OUT], b16)  # 64*[W1;W4]
            w2s_sb = singles.tile([128, HT, OUT], f8)
            w3s_sb = singles.tile([128, HT, OUT], f8)
            wsum_sb = singles.tile([128, HT, OUT], b16)  # 64*(W1+q2c*W4)
            wpk_sb = singles.tile([128, 3 * HT], f32)
            wq_sb = singles.tile([128, HT], b16)
            wcm8 = singles.tile([128, HT, 128], f8)      # 16*wc rank-1 fp8
            ones_mat = singles.tile([128, 128], b16)
            ones_col = singles.tile([128, 1], b16)
            ones_row_b = singles.tile([1, 128], b16)
            ones_row_f = singles.tile([1, 128], f32)
            bred_sb = singles.tile([1, OUT], b16)
            sq_sb = singles.tile([128, JT], f32)
            pt_sb = singles.tile([128, NCH, JT, CHW], b16)
            p8_sb = singles.tile([128, NCH, JT, CHW], f8)
            macc = singles.tile([128, C], b16)
            mall = singles.tile([128, C], b16)
            rl_sb = singles.tile([128, C], f32)          # 1/L
            escb = singles.tile([128, C], b16)
            ebc = singles.tile([128, C], b16)            # E = mall*escb
            ecol = singles.tile([128, CT], b16)          # E, c on parts
            bias_sb = singles.tile([128, OUT], b16)      # b_red bcast
            c2q8_sb = singles.tile([128, HT, C], f8)
            m38_sb = singles.tile([128, HT, C], f8)
            urow = singles.tile([1, H], f32)
            rs_sum = singles.tile([1, 1], f32)
            rs_col = singles.tile([128, 1], f32)
            q2cT = singles.tile([128, HT], f32)          # q2c, h on parts
            q2cs = singles.tile([128, HT], f32)

            # ---- loads (ordered for earliest PE start) -----------------
            nc.vector.memset(ones_mat, 1.0)
            nc.sync.dma_start(
                out=e1t_sb, in_=e1t_d.rearrange("(t p) j -> p t j", p=128)
            )
            nc.sync.dma_start(out=wpk_sb, in_=wpk_d)
            wcq_sb = wpk_sb[:, 0:HT]
            nc.vector.memset(ones_col, 1.0)
            nc.vector.memset(ones_row_b, 1.0)
            nc.vector.memset(ones_row_f, 1.0)
            nc.vector.memset(macc, 0.0)
            nc.vector.tensor_copy(wq_sb, wpk_sb[:, 2 * HT : 3 * HT])
            # HAM warm-up: keep the PE clocked while inputs stream in
            wps = ps_mm.tile([128, CHW], f32, tag="mm", name="warm")
            for _ in range(38):
                nc.tensor.matmul(wps[:, 0:128], ones_mat, ones_mat,
                                 start=True, stop=True)
            nc.vector.tensor_copy(rs_col, wps[:, 0:1])
            for ht in range(HT):
                nc.vector.tensor_scalar_mul(
                    e1w_sb[:, ht, :], e1t_sb[:, ht, :], wcq_sb[:, ht : ht + 1]
                )
            for ht in range(HT):
                # 16*wc broadcast as fp8 rank-1 stationary (host pre-scales)
                nc.vector.tensor_scalar_mul(
                    wcm8[:, ht, :], ones_mat, wpk_sb[:, HT + ht : HT + ht + 1]
                )
            e2t_r = e2t_d.rearrange("(t p) c -> p t c", p=128)
            for ch in range(NCH):
                csl = slice(ch * CHW, (ch + 1) * CHW)
                for ht in range(HT):
                    nc.sync.dma_start(out=e2t_sb[:, ht, csl], in_=e2t_r[:, ht, csl])
            nc.sync.dma_start(
                out=e18_sb, in_=e18_d.rearrange("(t p) h -> p t h", p=128)
            )
            nc.sync.dma_start(
                out=e2t8_sb, in_=e2t8_d.rearrange("(t p) c -> p t c", p=128)
            )
            nc.sync.dma_start(
                out=wrt14_sb, in_=wrt14_d.rearrange("(t p) o -> p t o", p=128)
            )
            nc.sync.dma_start(
                out=w2s_sb, in_=w2s_d.rearrange("(t p) o -> p t o", p=128)
            )
            nc.sync.dma_start(
                out=w3s_sb, in_=w3s_d.rearrange("(t p) o -> p t o", p=128)
            )
            nc.sync.dma_start(
                out=e2cm_sb, in_=e2cm_d.rearrange("(t p) h -> p t h", p=128)
            )
            nc.sync.dma_start(out=bred_sb, in_=bred_d)

            # ---- sq columns (tiny, feeds exp bias) ---------------------
            for jt in range(JT):
                ps = ps_mm.tile([128, CHW], f32, tag="mm")
                for ht in range(HT):
                    nc.tensor.matmul(
                        ps[:, 0:1],
                        e1t_sb[:, ht, jt * 128 : (jt + 1) * 128],
                        wq_sb[:, ht : ht + 1],
                        start=(ht == 0),
                        stop=(ht == HT - 1),
                    )
                nc.vector.tensor_copy(sq_sb[:, jt : jt + 1], ps[:, 0:1])

            # ---- phase A: bf16 scores, chunk-outer, L + P8 per chunk ---
            def a_chunk(ch):
                csl = slice(ch * CHW, (ch + 1) * CHW)
                for jt in range(JT):
                    ps = ps_mm.tile([128, CHW], f32, tag="mm", name=f"a{ch}_{jt}")
                    for ht in range(HT):
                        nc.tensor.matmul(
                            ps,
                            e1w_sb[:, ht, jt * 128 : (jt + 1) * 128],
                            e2t_sb[:, ht, csl],
                            start=(ht == 0),
                            stop=(ht == HT - 1),
                        )
                    nc.scalar.activation(
                        out=pt_sb[:, ch, jt, :], in_=ps, func=AF.Exp,
                        bias=sq_sb[:, jt : jt + 1], scale=1.0,
                    )
                    nc.vector.tensor_max(
                        macc[:, csl], macc[:, csl], pt_sb[:, ch, jt, :]
                    )
                nc.gpsimd.partition_all_reduce(
                    mall[:, csl], macc[:, csl], channels=128,
                    reduce_op=bass_isa.ReduceOp.max,
                )

            def l_chunk(ch):
                csl = slice(ch * CHW, (ch + 1) * CHW)
                lps = ps_mm.tile([128, CHW], f32, tag="mm", name=f"l{ch}")
                for jt in range(JT):
                    nc.tensor.matmul(
                        lps, ones_mat, pt_sb[:, ch, jt, :],
                        start=(jt == 0), stop=(jt == JT - 1),
                    )
                nc.vector.reciprocal_approx_fast(out=rl_sb[:, csl], in_=lps)
                for jt in range(JT):
                    nc.vector.scalar_tensor_tensor(
                        out=p8_sb[:, ch, jt, :],
                        in0=pt_sb[:, ch, jt, :],
                        scalar=float(PSCALE),
                        in1=rl_sb[:, csl],
                        op0=OP.mult, op1=OP.mult,
                    )

            def b_chunk(ch):
                # c2qT[h, csl] = sum_j e18[j,h] * P8[j,csl]; evict = scalar
                # copy with constant 1/PSCALE scale
                csl = slice(ch * CHW, (ch + 1) * CHW)
                for hg in range(2):
                    cps = [
                        ps_mm.tile([128, CHW], f32, tag="mm",
                                   name=f"b{ch}_{hg}_{i}")
                        for i in range(3)
                    ]
                    for i in range(3):
                        ht = hg * 3 + i
                        for jp in range(JT // 2):
                            nc.tensor.matmul(
                                cps[i],
                                e18_sb[:, 2 * jp : 2 * jp + 2,
                                       ht * 128 : (ht + 1) * 128],
                                p8_sb[:, ch, 2 * jp : 2 * jp + 2, :],
                                start=(jp == 0), stop=(jp == JT // 2 - 1),
                                perf_mode=DR,
                            )
                    for i in range(3):
                        ht = hg * 3 + i
                        nc.scalar.activation(
                            out=c2q8_sb[:, ht, csl], in_=cps[i],
                            func=AF.Copy, bias=0.0, scale=1.0 / PSCALE,
                        )

            def sc_chunk(ch):
                # fp8 DR rank-1: psum = 16*sc broadcast on all partitions
                csl = slice(ch * CHW, (ch + 1) * CHW)
                ps = ps_mm.tile([128, CHW], f32, tag="mm", name=f"sc{ch}")
                for hp in range(HT // 2):
                    nc.tensor.matmul(
                        ps,
                        wcm8[:, 2 * hp : 2 * hp + 2, :],
                        e2t8_sb[:, 2 * hp : 2 * hp + 2, csl],
                        start=(hp == 0), stop=(hp == HT // 2 - 1),
                        perf_mode=DR,
                    )
                nc.scalar.activation(
                    out=escb[:, csl], in_=ps, func=AF.Exp,
                    bias=0.0, scale=1.0 / SCS,
                )

            # ---- m38 = e2t * c2q8, chunk-pair wide DVE ops -------------
            def m38_pair(half):
                fsl = slice(half * 2 * CHW, (half + 1) * 2 * CHW)
                for ht in range(HT):
                    nc.vector.tensor_mul(
                        m38_sb[:, ht, fsl], e2t_sb[:, ht, fsl],
                        c2q8_sb[:, ht, fsl],
                    )

            # E row half -> c-partitions (via DRAM bounce); q2c chain is
            # gated on mall, so pipeline the first half behind A's tail
            def ecol_half(half):
                fsl = slice(half * 2 * CHW, (half + 1) * 2 * CHW)
                nc.vector.tensor_mul(
                    ebc[0:1, fsl], mall[0:1, fsl], escb[0:1, fsl]
                )
                nc.sync.dma_start(out=ebcrow_d[:, fsl], in_=ebc[0:1, fsl])
                nc.sync.dma_start(
                    out=ecol[:, half * 8 : (half + 1) * 8],
                    in_=ebcrow_d[:, fsl].rearrange("1 (t p) -> p t", p=128),
                )

            a_chunk(0)
            l_chunk(0)
            a_chunk(1)
            l_chunk(1)
            b_chunk(0)
            sc_chunk(0)
            a_chunk(2)
            l_chunk(2)
            b_chunk(1)
            sc_chunk(1)
            a_chunk(3)
            l_chunk(3)
            sc_chunk(2)
            sc_chunk(3)
            b_chunk(2)
            b_chunk(3)
            ecol_half(0)
            m38_pair(0)
            ecol_half(1)
            m38_pair(1)

            # bias broadcast (k=1 matmul) for the DVE eviction add
            bps = ps_mm.tile([128, CHW], f32, tag="mm", name="bias")
            nc.tensor.matmul(
                bps[:, 0:OUT], ones_row_b, bred_sb, start=True, stop=True
            )
            nc.vector.tensor_copy(bias_sb, bps[:, 0:OUT])

            # ---- q2c on PE: u = ecol.T @ e2cm, S = sum(E) --------------
            ups1 = ps_mm.tile([128, CHW], f32, tag="mm", name="u1")
            ups2 = ps_mm.tile([128, CHW], f32, tag="mm", name="u2")
            sps = ps_mm.tile([128, CHW], f32, tag="mm", name="sS")
            for ct in range(CT):
                nc.tensor.matmul(
                    ups1[0:1, 0:CHW], ecol[:, ct : ct + 1],
                    e2cm_sb[:, ct, 0:CHW],
                    start=(ct == 0), stop=(ct == CT - 1),
                )
                nc.tensor.matmul(
                    ups2[0:1, 0 : H - CHW], ecol[:, ct : ct + 1],
                    e2cm_sb[:, ct, CHW:H],
                    start=(ct == 0), stop=(ct == CT - 1),
                )
                nc.tensor.matmul(
                    sps[0:1, 0:1], ecol[:, ct : ct + 1], ones_col,
                    start=(ct == 0), stop=(ct == CT - 1),
                )
            nc.vector.tensor_copy(urow[:, 0:CHW], ups1[0:1, 0:CHW])
            nc.vector.tensor_copy(urow[:, CHW:H], ups2[0:1, 0 : H - CHW])
            nc.vector.reciprocal_approx_fast(out=rs_sum, in_=sps[0:1, 0:1])
            nc.sync.dma_start(out=urow_d, in_=urow)
            nc.sync.dma_start(
                out=q2cT, in_=urow_d.rearrange("1 (t p) -> p t", p=128)
            )
            rps = ps_mm.tile([128, CHW], f32, tag="mm", name="rsb")
            nc.tensor.matmul(
                rps[:, 0:1], ones_row_f, rs_sum, start=True, stop=True
            )
            nc.vector.tensor_copy(rs_col, rps[:, 0:1])
            nc.vector.tensor_scalar_mul(q2cs, q2cT, rs_col)
            # wsum64 = 64*W1 + q2c * 64*W4, one fused DVE op per ht
            for ht in range(HT):
                nc.vector.scalar_tensor_tensor(
                    out=wsum_sb[:, ht, :],
                    in0=wrt14_sb[:, 6 + ht, :],
                    scalar=q2cs[:, ht : ht + 1],
                    in1=wrt14_sb[:, ht, :],
                    op0=OP.mult, op1=OP.add,
                )

            # ---- reduction: R23 (fp8 DR) + R1 (bf16) + bias, fused -----
            def r23(ct):
                tsl = slice(ct * 128, (ct + 1) * 128)
                ops = ps_out.tile([128, OUT], f32, tag="out", name=f"o{ct}")
                for hp in range(HT // 2):
                    nc.tensor.matmul(
                        ops,
                        c2q8_sb[:, 2 * hp : 2 * hp + 2, tsl],
                        w2s_sb[:, 2 * hp : 2 * hp + 2, :],
                        start=(hp == 0), stop=False,
                        perf_mode=DR,
                    )
                for hp in range(HT // 2):
                    nc.tensor.matmul(
                        ops,
                        m38_sb[:, 2 * hp : 2 * hp + 2, tsl],
                        w3s_sb[:, 2 * hp : 2 * hp + 2, :],
                        start=False, stop=False,
                        perf_mode=DR,
                    )
                return ops

            def r1(ct, ops):
                tsl = slice(ct * 128, (ct + 1) * 128)
                for ht in range(HT):
                    nc.tensor.matmul(
                        ops, e2t_sb[:, ht, tsl], wsum_sb[:, ht, :],
                        start=False, stop=(ht == HT - 1),
                    )
                od = odp.tile([128, OUT], b16, tag="od", name=f"od{ct}")
                nc.vector.scalar_tensor_tensor(
                    out=od, in0=ops, scalar=1.0 / WS, in1=bias_sb,
                    op0=OP.mult, op1=OP.add,
                )
                nc.sync.dma_start(out=out_d[tsl, :], in_=od)

            WINDOW = 3
            open_ps = {}
            for ct in range(WINDOW):
                open_ps[ct] = r23(ct)
            for ct in range(CT):
                r1(ct, open_ps.pop(ct))
                nxt = ct + WINDOW
                if nxt < CT:
                    open_ps[nxt] = r23(nxt)

    nc.compile()
    return nc


def _get_nc():
    if "nc" not in _CACHE:
        _CACHE["nc"] = _build_bass()
    return _CACHE["nc"]


def _in_maps(emb1, emb2, w_c, b_c, w_q, b_q, w_cq, b_cq, w_red, b_red):
    # host-side sharding + layout only: batch split, transposes, casts
    emb1 = np.asarray(emb1, np.float32)
    emb2 = np.asarray(emb2, np.float32)
    wcq = np.asarray(w_cq, np.float32).reshape(HT, 128).T
    wc = np.asarray(w_c, np.float32).reshape(HT, 128).T * SCS
    wq = np.asarray(w_q, np.float32).reshape(HT, 128).T
    wpk = np.ascontiguousarray(np.concatenate([wcq, wc, wq], axis=1))
    wrt = np.ascontiguousarray(np.asarray(w_red, np.float32).T)  # (4H, OUT)
    wrt14 = np.concatenate([wrt[0:H] * WS, wrt[3 * H : 4 * H] * WS], axis=0)
    w2s = (wrt[H : 2 * H] * WS).astype(f8_np)
    w3s = (wrt[2 * H : 3 * H] * WS).astype(f8_np)
    bred = np.asarray(b_red, np.float32).reshape(1, OUT).astype(bf16)
    maps = []
    for b in range(B):
        e2t = np.ascontiguousarray(emb2[b].T)
        maps.append(
            {
                "e1t": np.ascontiguousarray(emb1[b].T).astype(bf16),
                "e18": emb1[b].astype(f8_np),
                "e2t": e2t.astype(bf16),
                "e2t8": e2t.astype(f8_np),
                "e2cm": emb2[b].astype(bf16),
                "wrt14": wrt14.astype(bf16),
                "w2s": w2s,
                "w3s": w3s,
                "wpk": wpk,
                "bred": bred,
            }
        )
    return maps


def run(inputs, trace=False):
    from concourse.bass_utils import run_bass_kernel_spmd

    nc = _get_nc()
    maps = _in_maps(**inputs)
    res = run_bass_kernel_spmd(nc, maps, list(range(B)), trace=trace)
    out = np.stack(
        [res.results[b]["out"].astype(np.float32) for b in range(B)], axis=0
    )
    return out, res


def kernel(**inputs) -> np.ndarray:
    out, _ = run(inputs, trace=False)
    return out
